# revision 22
# baseline (speedup 1.0000x reference)
"""Trainium2 Bass kernel for GPT-style attention block (B=2, S=2048, D=1024, H=16).

Sharding: tensor-parallel over heads, 2 heads per core (8 cores).
Each core computes qkv for its heads, causal softmax attention, its partial
output projection (contracting only its 128 head-dims); host sums the 8
partial projections (+ b_proj) and concatenates head-sharded attn/present.

Precision: fp16 matmul operands (1 cyc/row, fp32 PSUM accumulation); the
softmax runs in fp32 with fp16-rounded outputs (~5e-4 rel err).
Structure: x arrives host-pre-transposed; scores are computed twice — once
row-major for the attn output (exp + normalize, exact causal widths), once
transposed (k.T@q) feeding attn@v directly. Softmax row-sums fall out of
the attn@v matmul via a ones-column appended to the stationary v tiles.
Causal upper region relies on pre-zeroed DRAM outputs.
"""
import numpy as np

import concourse.bacc as bacc
import concourse.mybir as mybir
import concourse.tile as tile
from concourse.bass_utils import run_bass_kernel_spmd
from concourse.masks import make_identity

B, S, D, H = 2, 2048, 1024, 16
HD = D // H            # 64
N_CORES = 8
HPC = H // N_CORES     # 2 heads per core
PD = HPC * HD          # 128 partition dims per core
F32 = mybir.dt.float32
F16 = mybir.dt.float16
AF = mybir.ActivationFunctionType

_CACHE = {}
_last_in_maps = None


def _build():
    nc = bacc.Bacc(None, target_bir_lowering=False)

    xt_ext = nc.declare_dram_parameter("x16t", [D, B * S], F16, isOutput=False)
    wqkv_ext = nc.declare_dram_parameter("wqkv16", [D, 3 * PD], F16, isOutput=False)
    bqkv_ext = nc.declare_dram_parameter("bqkv", [3, PD], F32, isOutput=False)
    wp_ext = nc.declare_dram_parameter("wp16", [PD, D], F16, isOutput=False)
    attn_ext = nc.declare_dram_parameter("attn", [B, HPC, S, S], F32, isOutput=True)
    pres_ext = nc.declare_dram_parameter("present", [2, B, HPC, S, HD], F32, isOutput=True)
    apart_ext = nc.declare_dram_parameter("a_part", [B, S, D], F32, isOutput=True)

    with tile.TileContext(nc) as tc:
        with (
            tc.tile_pool(name="cst", bufs=1) as cst,
            tc.tile_pool(name="sb", bufs=1) as sb,
            tc.tile_pool(name="ps", bufs=1, space="PSUM") as ps,
        ):
            # ---- constants ----
            ident = cst.tile([128, 128], F32)
            make_identity(nc, ident[:])
            ident16 = cst.tile([128, 128], F16)
            nc.vector.tensor_copy(ident16[:], ident[:])

            # row-major diag-block mask: keep where col <= row
            trimask = cst.tile([128, 128], F32)
            nc.gpsimd.memset(trimask[:], 0.0)
            nc.gpsimd.affine_select(
                out=trimask[:], in_=trimask[:],
                compare_op=mybir.AluOpType.is_ge,
                fill=-1e9, base=0,
                pattern=[[-1, 128]], channel_multiplier=1,
            )
            # transposed diag-block mask: keep where qrow-offset >= kcol-offset
            trimaskT = cst.tile([128, 128], F32)
            nc.gpsimd.memset(trimaskT[:], 0.0)
            nc.gpsimd.affine_select(
                out=trimaskT[:], in_=trimaskT[:],
                compare_op=mybir.AluOpType.is_ge,
                fill=-1e9, base=0,
                pattern=[[1, 128]], channel_multiplier=-1,
            )

            # qkv weights: 8 k-blocks x 3 col-tiles (fp16, direct DMA)
            wsb = [[None] * 3 for _ in range(8)]
            for k in range(8):
                for t in range(3):
                    wr = cst.tile([128, 128], F16, name=f"w{k}_{t}")
                    nc.sync.dma_start(wr[:], wqkv_ext[128 * k:128 * (k + 1), 128 * t:128 * (t + 1)])
                    wsb[k][t] = wr

            bq_sb = cst.tile([128, 3], F32)
            for t in range(3):
                nc.sync.dma_start(bq_sb[:, t:t + 1], bqkv_ext[t][:, None])

            wp16 = cst.tile([PD, D], F16)
            nc.sync.dma_start(wp16[:], wp_ext[:])

            for b in range(B):
                # ---- qkv projection (transposed activations direct from DRAM) ----
                qT = sb.tile([128, S], F16, tag="qT", bufs=2, name=f"qT{b}")
                kT = sb.tile([128, S], F16, tag="kT", bufs=2, name=f"kT{b}")
                vbigs = []
                for n in range(2):              # 1024-wide s-chunks
                    xts = []
                    for k in range(8):
                        xt = sb.tile([128, 1024], F16, tag="xt", bufs=12, name=f"xt{b}_{n}_{k}")
                        nc.sync.dma_start(
                            xt[:], xt_ext[128 * k:128 * (k + 1),
                                          2048 * b + 1024 * n:2048 * b + 1024 * (n + 1)])
                        xts.append(xt)

                    for t in range(3):
                        psq = ps.tile([128, 1024], F32, tag="mmE", bufs=2, name=f"pq{b}_{n}_{t}")
                        for k in range(8):
                            for half in range(2):
                                nc.tensor.matmul(psq[:, 512 * half:512 * (half + 1)],
                                                 wsb[k][t][:],
                                                 xts[k][:, 512 * half:512 * (half + 1)],
                                                 start=(k == 0), stop=(k == 7))
                        if t == 0:
                            dst = qT[:, 1024 * n:1024 * (n + 1)]
                        elif t == 1:
                            dst = kT[:, 1024 * n:1024 * (n + 1)]
                        else:
                            vt = sb.tile([128, 1024], F16, tag="vt", bufs=3, name=f"vt{b}_{n}")
                            dst = vt[:]
                        nc.vector.tensor_scalar_add(dst, psq[:], bq_sb[:, t:t + 1])

                    # k/v row-major (present outputs via casting DMA; v + ones
                    # columns also feed attn@v)
                    psk = ps.tile([128, 1024], F16, tag="mmT", bufs=2, name=f"psk{b}_{n}")
                    for r in range(8):
                        nc.tensor.transpose(psk[:, 128 * r:128 * (r + 1)],
                                            kT[:, 1024 * n + 128 * r:1024 * n + 128 * (r + 1)],
                                            ident16[:])
                    kbig = sb.tile([128, 1024], F16, tag="kbig", bufs=3, name=f"kb{b}_{n}")
                    nc.vector.tensor_copy(kbig[:], psk[:])
                    for hh2 in range(2):
                        nc.gpsimd.dma_start(
                            pres_ext[0, b, hh2, 1024 * n:1024 * (n + 1), :]
                            .rearrange("(r p) d -> p r d", p=128),
                            kbig[:].rearrange("p (r h d) -> p r h d", r=8, h=2)[:, :, hh2, :])
                    psv = ps.tile([128, 1024], F16, tag="mmT", bufs=2, name=f"psv{b}_{n}")
                    for r in range(8):
                        nc.tensor.transpose(psv[:, 128 * r:128 * (r + 1)],
                                            vt[:, 128 * r:128 * (r + 1)], ident16[:])
                    # vbig layout per 130-col block r: [v_h0(64) | ones | v_h1(64) | ones]
                    vbig = sb.tile([128, 1040], F16, tag="vbig", bufs=4, name=f"vb{b}_{n}")
                    nc.vector.tensor_copy(
                        vbig[:].rearrange("p (k c) -> p k c", c=65)[:, :, 0:64],
                        psv[:].rearrange("p (k d) -> p k d", d=64))
                    nc.gpsimd.memset(
                        vbig[:].rearrange("p (k c) -> p k c", c=65)[:, :, 64:65], 1.0)
                    for hh2 in range(2):
                        nc.gpsimd.dma_start(
                            pres_ext[1, b, hh2, 1024 * n:1024 * (n + 1), :]
                            .rearrange("(r p) d -> p r d", p=128),
                            vbig[:].rearrange("p (r h c) -> p r h c", h=2, c=65)[:, :, hh2, 0:64])
                    vbigs.append(vbig)

                # ---- attention ----
                avT = sb.tile([128, S], F16, tag="avT", bufs=2, name=f"avT{b}")
                for g in range(4):
                    for hh in range(2):
                        hs = 64 * hh
                        # transposed side: scoresT -> expT -> attn@v (+row sums)
                        avp = ps.tile([65, 512], F32, tag="av", bufs=2,
                                      name=f"avp{b}_{hh}_{g}")
                        njs = 4 * (g + 1)
                        for j in range(njs):
                            qoff = max(0, 128 * (j - 4 * g))
                            nw = 512 - qoff
                            pssT = ps.tile([128, 512], F32, tag="mmT", bufs=2,
                                           name=f"psT{b}_{hh}_{g}_{j}")
                            nc.tensor.matmul(
                                pssT[:, :nw],
                                kT[hs:hs + 64, 128 * j:128 * (j + 1)],
                                qT[hs:hs + 64, 512 * g + qoff:512 * (g + 1)],
                                start=True, stop=True)
                            if j >= 4 * g:
                                nc.vector.tensor_add(pssT[:, 0:128], pssT[:, 0:128],
                                                     trimaskT[:])
                            ET = sb.tile([128, 512], F16, tag="ET", bufs=6,
                                         name=f"ET{b}_{hh}_{g}_{j}")
                            nc.scalar.activation(out=ET[:, :nw], in_=pssT[:, :nw],
                                                 func=AF.Exp, scale=0.125)
                            vb = vbigs[j // 8]
                            lhsv = vb[:, 130 * (j % 8) + 65 * hh:130 * (j % 8) + 65 * hh + 65]
                            nc.tensor.matmul(avp[:, qoff:512], lhsv, ET[:, :nw],
                                             start=(j == 0), stop=(j == njs - 1))
                        # row sums -> per-row reciprocals -> broadcast numerators
                        rsT = sb.tile([1, 512], F32, tag="rsT", bufs=3,
                                      name=f"rt{b}_{hh}_{g}")
                        nc.vector.tensor_copy(rsT[:], avp[64:65, :])
                        stgp = ps.tile([128, 4], F32, tag="mmT", bufs=2,
                                       name=f"sg{b}_{hh}_{g}")
                        for r in range(4):
                            nc.tensor.transpose(stgp[:, r:r + 1],
                                                rsT[0:1, 128 * r:128 * (r + 1)],
                                                ident[0:1, 0:1])
                        stage = sb.tile([128, 4], F32, tag="stg", bufs=4,
                                        name=f"st{b}_{hh}_{g}")
                        nc.vector.reciprocal(stage[:], stgp[:])
                        rcpTp = ps.tile([1, 512], F32, tag="mmT", bufs=2,
                                        name=f"rp{b}_{hh}_{g}")
                        for r in range(4):
                            nc.tensor.transpose(rcpTp[0:1, 128 * r:128 * (r + 1)],
                                                stage[:, r:r + 1], ident[:])
                        rcpT = sb.tile([1, 512], F32, tag="rcpT", bufs=3,
                                       name=f"rr{b}_{hh}_{g}")
                        nc.vector.tensor_copy(rcpT[:], rcpTp[0:1, :])
                        rbs = sb.tile([64, 512], F32, tag="rb", bufs=3,
                                      name=f"rb{b}_{hh}_{g}")
                        nc.gpsimd.partition_broadcast(rbs[:], rcpT[:])
                        nc.vector.tensor_mul(avT[hs:hs + 64, 512 * g:512 * (g + 1)],
                                             avp[0:64, :], rbs[:])
                        # row-major side: exp with exact causal widths -> attn out
                        for r in range(4):
                            i = 4 * g + r
                            widE = 128 * (i + 1)
                            E = sb.tile([128, 2048], F16, tag="E", bufs=8,
                                        name=f"E{b}_{hh}_{i}")
                            for p in range((widE + 1023) // 1024):
                                wp_ = min(1024, widE - 1024 * p)
                                pss = ps.tile([128, 1024], F32, tag="mmE", bufs=2,
                                              name=f"pss{b}_{hh}_{i}_{p}")
                                for c0 in range(0, wp_, 512):
                                    cw = min(512, wp_ - c0)
                                    nc.tensor.matmul(
                                        pss[:, c0:c0 + cw],
                                        qT[hs:hs + 64, 128 * i:128 * (i + 1)],
                                        kT[hs:hs + 64, 1024 * p + c0:1024 * p + c0 + cw],
                                        start=True, stop=True)
                                if 1024 * p + wp_ == widE:
                                    nc.vector.tensor_add(pss[:, wp_ - 128:wp_],
                                                         pss[:, wp_ - 128:wp_], trimask[:])
                                nc.scalar.activation(out=E[:, 1024 * p:1024 * p + wp_],
                                                     in_=pss[:, :wp_],
                                                     func=AF.Exp, scale=0.125)
                            nc.vector.tensor_scalar_mul(E[:, :widE], E[:, :widE],
                                                        stage[:, r:r + 1])
                            nc.gpsimd.dma_start(
                                attn_ext[b, hh, 128 * i:128 * (i + 1), 0:widE],
                                E[:, :widE])
                    # ---- partial projection for this row group (fp16) ----
                    for mi in range(4):
                        m = 4 * g + mi
                        psp = ps.tile([128, 1024], F32, tag="mmE", bufs=2,
                                      name=f"pp{b}_{g}_{mi}")
                        for nn2 in range(2):
                            nc.tensor.matmul(psp[:, 512 * nn2:512 * (nn2 + 1)],
                                             avT[:, 128 * m:128 * (m + 1)],
                                             wp16[:, 512 * nn2:512 * (nn2 + 1)],
                                             start=True, stop=True)
                        ao = sb.tile([128, 1024], F32, tag="ao", bufs=4,
                                     name=f"ao{b}_{g}_{mi}")
                        if mi % 2 == 0:
                            nc.scalar.copy(ao[:], psp[:])
                        else:
                            nc.vector.tensor_copy(ao[:], psp[:])
                        nc.sync.dma_start(apart_ext[b, 128 * m:128 * (m + 1), :], ao[:])
    nc.compile()
    return nc


def _get_nc():
    if "nc" not in _CACHE:
        _CACHE["nc"] = _build()
    return _CACHE["nc"]


def kernel(x, w_attn, b_attn, w_proj, b_proj):
    global _last_in_maps
    x = np.asarray(x, dtype=np.float32)
    w_attn = np.asarray(w_attn, dtype=np.float32)
    b_attn = np.asarray(b_attn, dtype=np.float32)
    w_proj = np.asarray(w_proj, dtype=np.float32)
    b_proj = np.asarray(b_proj, dtype=np.float32)

    nc = _get_nc()
    x16t = np.ascontiguousarray(x.reshape(B * S, D).astype(np.float16).T)
    in_maps = []
    for c in range(N_CORES):
        lo, hi = PD * c, PD * (c + 1)
        wqkv = np.ascontiguousarray(np.concatenate(
            [w_attn[:, lo:hi], w_attn[:, D + lo:D + hi], w_attn[:, 2 * D + lo:2 * D + hi]],
            axis=1).astype(np.float16))
        bqkv = np.ascontiguousarray(np.stack(
            [b_attn[lo:hi], b_attn[D + lo:D + hi], b_attn[2 * D + lo:2 * D + hi]]))
        in_maps.append({
            "x16t": x16t,
            "wqkv16": wqkv,
            "bqkv": bqkv,
            "wp16": np.ascontiguousarray(w_proj[lo:hi, :].astype(np.float16)),
        })

    _last_in_maps = in_maps
    res = run_bass_kernel_spmd(nc, in_maps, list(range(N_CORES)))
    rs = res.results

    attn = np.concatenate([r["attn"] for r in rs], axis=1)          # [B, H, S, S]
    present = np.concatenate([r["present"] for r in rs], axis=2)    # [2, B, H, S, HD]
    a = rs[0]["a_part"]
    for r in rs[1:]:
        a = a + r["a_part"]
    a = a + b_proj
    return a, present, attn


# revision 23
# speedup vs baseline: 1.0031x; 1.0031x over previous
"""Trainium2 Bass kernel for GPT-style attention block (B=2, S=2048, D=1024, H=16).

Sharding: tensor-parallel over heads, 2 heads per core (8 cores).
Each core computes qkv for its heads, causal softmax attention, its partial
output projection (contracting only its 128 head-dims); host sums the 8
partial projections (+ b_proj) and concatenates head-sharded attn/present.

Precision: fp16 matmul operands (1 cyc/row, fp32 PSUM accumulation); the
softmax runs in fp32 with fp16-rounded outputs (~5e-4 rel err).
Structure: x arrives host-pre-transposed; scores are computed twice — once
row-major for the attn output (exp + normalize, exact causal widths), once
transposed (k.T@q) feeding attn@v directly. Softmax row-sums fall out of
the attn@v matmul via a ones-column appended to the stationary v tiles.
Causal upper region relies on pre-zeroed DRAM outputs.
"""
import numpy as np

import concourse.bacc as bacc
import concourse.mybir as mybir
import concourse.tile as tile
from concourse.bass_utils import run_bass_kernel_spmd
from concourse.masks import make_identity

B, S, D, H = 2, 2048, 1024, 16
HD = D // H            # 64
N_CORES = 8
HPC = H // N_CORES     # 2 heads per core
PD = HPC * HD          # 128 partition dims per core
F32 = mybir.dt.float32
F16 = mybir.dt.float16
AF = mybir.ActivationFunctionType

_CACHE = {}
_last_in_maps = None


def _build():
    nc = bacc.Bacc(None, target_bir_lowering=False)

    xt_ext = nc.declare_dram_parameter("x16t", [D, B * S], F16, isOutput=False)
    wqkv_ext = nc.declare_dram_parameter("wqkv16", [D, 3 * PD], F16, isOutput=False)
    bqkv_ext = nc.declare_dram_parameter("bqkv", [3, PD], F32, isOutput=False)
    wp_ext = nc.declare_dram_parameter("wp16", [PD, D], F16, isOutput=False)
    attn_ext = nc.declare_dram_parameter("attn", [B, HPC, S, S], F32, isOutput=True)
    pres_ext = nc.declare_dram_parameter("present", [2, B, HPC, S, HD], F32, isOutput=True)
    apart_ext = nc.declare_dram_parameter("a_part", [B, S, D], F32, isOutput=True)

    with tile.TileContext(nc) as tc:
        with (
            tc.tile_pool(name="cst", bufs=1) as cst,
            tc.tile_pool(name="sb", bufs=1) as sb,
            tc.tile_pool(name="ps", bufs=1, space="PSUM") as ps,
        ):
            # ---- constants ----
            ident = cst.tile([128, 128], F32)
            make_identity(nc, ident[:])
            ident16 = cst.tile([128, 128], F16)
            nc.vector.tensor_copy(ident16[:], ident[:])

            # row-major diag-block mask: keep where col <= row
            trimask = cst.tile([128, 128], F32)
            nc.gpsimd.memset(trimask[:], 0.0)
            nc.gpsimd.affine_select(
                out=trimask[:], in_=trimask[:],
                compare_op=mybir.AluOpType.is_ge,
                fill=-1e9, base=0,
                pattern=[[-1, 128]], channel_multiplier=1,
            )
            # transposed diag-block mask: keep where qrow-offset >= kcol-offset
            trimaskT = cst.tile([128, 128], F32)
            nc.gpsimd.memset(trimaskT[:], 0.0)
            nc.gpsimd.affine_select(
                out=trimaskT[:], in_=trimaskT[:],
                compare_op=mybir.AluOpType.is_ge,
                fill=-1e9, base=0,
                pattern=[[1, 128]], channel_multiplier=-1,
            )

            # qkv weights: 8 k-blocks x 3 col-tiles (fp16, direct DMA)
            wsb = [[None] * 3 for _ in range(8)]
            for k in range(8):
                for t in range(3):
                    wr = cst.tile([128, 128], F16, name=f"w{k}_{t}")
                    nc.sync.dma_start(wr[:], wqkv_ext[128 * k:128 * (k + 1), 128 * t:128 * (t + 1)])
                    wsb[k][t] = wr

            bq_sb = cst.tile([128, 3], F32)
            for t in range(3):
                nc.sync.dma_start(bq_sb[:, t:t + 1], bqkv_ext[t][:, None])

            wp16 = cst.tile([PD, D], F16)
            nc.sync.dma_start(wp16[:], wp_ext[:])

            for b in range(B):
                # ---- qkv projection (transposed activations direct from DRAM) ----
                qT = sb.tile([128, S], F16, tag="qT", bufs=2, name=f"qT{b}")
                kT = sb.tile([128, S], F16, tag="kT", bufs=2, name=f"kT{b}")
                vbigs = []
                for n in range(2):              # 1024-wide s-chunks
                    xts = []
                    for k in range(8):
                        xt = sb.tile([128, 1024], F16, tag="xt", bufs=12, name=f"xt{b}_{n}_{k}")
                        nc.sync.dma_start(
                            xt[:], xt_ext[128 * k:128 * (k + 1),
                                          2048 * b + 1024 * n:2048 * b + 1024 * (n + 1)])
                        xts.append(xt)

                    for t in range(3):
                        psq = ps.tile([128, 1024], F32, tag="mmE", bufs=2, name=f"pq{b}_{n}_{t}")
                        for k in range(8):
                            for half in range(2):
                                nc.tensor.matmul(psq[:, 512 * half:512 * (half + 1)],
                                                 wsb[k][t][:],
                                                 xts[k][:, 512 * half:512 * (half + 1)],
                                                 start=(k == 0), stop=(k == 7))
                        if t == 0:
                            dst = qT[:, 1024 * n:1024 * (n + 1)]
                        elif t == 1:
                            dst = kT[:, 1024 * n:1024 * (n + 1)]
                        else:
                            vt = sb.tile([128, 1024], F16, tag="vt", bufs=3, name=f"vt{b}_{n}")
                            dst = vt[:]
                        nc.vector.tensor_scalar_add(dst, psq[:], bq_sb[:, t:t + 1])

                    # k/v row-major (present outputs via casting DMA; v + ones
                    # columns also feed attn@v)
                    psk = ps.tile([128, 1024], F16, tag="mmT", bufs=2, name=f"psk{b}_{n}")
                    for r in range(8):
                        nc.tensor.transpose(psk[:, 128 * r:128 * (r + 1)],
                                            kT[:, 1024 * n + 128 * r:1024 * n + 128 * (r + 1)],
                                            ident16[:])
                    kbig = sb.tile([128, 1024], F32, tag="kbig", bufs=3, name=f"kb{b}_{n}")
                    nc.vector.tensor_copy(kbig[:], psk[:])
                    for hh2 in range(2):
                        nc.sync.dma_start(
                            pres_ext[0, b, hh2, 1024 * n:1024 * (n + 1), :]
                            .rearrange("(r p) d -> p r d", p=128),
                            kbig[:].rearrange("p (r h d) -> p r h d", r=8, h=2)[:, :, hh2, :])
                    psv = ps.tile([128, 1024], F16, tag="mmT", bufs=2, name=f"psv{b}_{n}")
                    for r in range(8):
                        nc.tensor.transpose(psv[:, 128 * r:128 * (r + 1)],
                                            vt[:, 128 * r:128 * (r + 1)], ident16[:])
                    # vbig layout per 130-col block r: [v_h0(64) | ones | v_h1(64) | ones]
                    vbig = sb.tile([128, 1040], F16, tag="vbig", bufs=4, name=f"vb{b}_{n}")
                    nc.vector.tensor_copy(
                        vbig[:].rearrange("p (k c) -> p k c", c=65)[:, :, 0:64],
                        psv[:].rearrange("p (k d) -> p k d", d=64))
                    nc.vector.memset(
                        vbig[:].rearrange("p (k c) -> p k c", c=65)[:, :, 64:65], 1.0)
                    for hh2 in range(2):
                        nc.gpsimd.dma_start(
                            pres_ext[1, b, hh2, 1024 * n:1024 * (n + 1), :]
                            .rearrange("(r p) d -> p r d", p=128),
                            vbig[:].rearrange("p (r h c) -> p r h c", h=2, c=65)[:, :, hh2, 0:64])
                    vbigs.append(vbig)

                # ---- attention ----
                avT = sb.tile([128, S], F16, tag="avT", bufs=2, name=f"avT{b}")
                for g in range(4):
                    for hh in range(2):
                        hs = 64 * hh
                        # transposed side: scoresT -> expT -> attn@v (+row sums)
                        avp = ps.tile([65, 512], F32, tag="av", bufs=2,
                                      name=f"avp{b}_{hh}_{g}")
                        njs = 4 * (g + 1)
                        for j in range(njs):
                            qoff = max(0, 128 * (j - 4 * g))
                            nw = 512 - qoff
                            pssT = ps.tile([128, 512], F32, tag="mmT", bufs=2,
                                           name=f"psT{b}_{hh}_{g}_{j}")
                            nc.tensor.matmul(
                                pssT[:, :nw],
                                kT[hs:hs + 64, 128 * j:128 * (j + 1)],
                                qT[hs:hs + 64, 512 * g + qoff:512 * (g + 1)],
                                start=True, stop=True)
                            if j >= 4 * g:
                                nc.vector.tensor_add(pssT[:, 0:128], pssT[:, 0:128],
                                                     trimaskT[:])
                            ET = sb.tile([128, 512], F16, tag="ET", bufs=8,
                                         name=f"ET{b}_{hh}_{g}_{j}")
                            nc.scalar.activation(out=ET[:, :nw], in_=pssT[:, :nw],
                                                 func=AF.Exp, scale=0.125)
                            vb = vbigs[j // 8]
                            lhsv = vb[:, 130 * (j % 8) + 65 * hh:130 * (j % 8) + 65 * hh + 65]
                            nc.tensor.matmul(avp[:, qoff:512], lhsv, ET[:, :nw],
                                             start=(j == 0), stop=(j == njs - 1))
                        # row sums -> per-row reciprocals -> broadcast numerators
                        rsT = sb.tile([1, 512], F32, tag="rsT", bufs=3,
                                      name=f"rt{b}_{hh}_{g}")
                        nc.vector.tensor_copy(rsT[:], avp[64:65, :])
                        stgp = ps.tile([128, 4], F32, tag="mmT", bufs=2,
                                       name=f"sg{b}_{hh}_{g}")
                        for r in range(4):
                            nc.tensor.transpose(stgp[:, r:r + 1],
                                                rsT[0:1, 128 * r:128 * (r + 1)],
                                                ident[0:1, 0:1])
                        stage = sb.tile([128, 4], F32, tag="stg", bufs=4,
                                        name=f"st{b}_{hh}_{g}")
                        nc.vector.reciprocal(stage[:], stgp[:])
                        rcpTp = ps.tile([1, 512], F32, tag="mmT", bufs=2,
                                        name=f"rp{b}_{hh}_{g}")
                        for r in range(4):
                            nc.tensor.transpose(rcpTp[0:1, 128 * r:128 * (r + 1)],
                                                stage[:, r:r + 1], ident[:])
                        rcpT = sb.tile([1, 512], F32, tag="rcpT", bufs=3,
                                       name=f"rr{b}_{hh}_{g}")
                        nc.vector.tensor_copy(rcpT[:], rcpTp[0:1, :])
                        rbs = sb.tile([64, 512], F32, tag="rb", bufs=3,
                                      name=f"rb{b}_{hh}_{g}")
                        nc.gpsimd.partition_broadcast(rbs[:], rcpT[:])
                        nc.vector.tensor_mul(avT[hs:hs + 64, 512 * g:512 * (g + 1)],
                                             avp[0:64, :], rbs[:])
                        # row-major side: exp with exact causal widths -> attn out
                        for r in range(4):
                            i = 4 * g + r
                            widE = 128 * (i + 1)
                            E = sb.tile([128, 2048], F16, tag="E", bufs=9,
                                        name=f"E{b}_{hh}_{i}")
                            for p in range((widE + 1023) // 1024):
                                wp_ = min(1024, widE - 1024 * p)
                                pss = ps.tile([128, 1024], F32, tag="mmE", bufs=2,
                                              name=f"pss{b}_{hh}_{i}_{p}")
                                for c0 in range(0, wp_, 512):
                                    cw = min(512, wp_ - c0)
                                    nc.tensor.matmul(
                                        pss[:, c0:c0 + cw],
                                        qT[hs:hs + 64, 128 * i:128 * (i + 1)],
                                        kT[hs:hs + 64, 1024 * p + c0:1024 * p + c0 + cw],
                                        start=True, stop=True)
                                if 1024 * p + wp_ == widE:
                                    nc.vector.tensor_add(pss[:, wp_ - 128:wp_],
                                                         pss[:, wp_ - 128:wp_], trimask[:])
                                nc.scalar.activation(out=E[:, 1024 * p:1024 * p + wp_],
                                                     in_=pss[:, :wp_],
                                                     func=AF.Exp, scale=0.125)
                            nc.vector.tensor_scalar_mul(E[:, :widE], E[:, :widE],
                                                        stage[:, r:r + 1])
                            nc.gpsimd.dma_start(
                                attn_ext[b, hh, 128 * i:128 * (i + 1), 0:widE],
                                E[:, :widE])
                    # ---- partial projection for this row group (fp16) ----
                    for mi in range(4):
                        m = 4 * g + mi
                        psp = ps.tile([128, 1024], F32, tag="mmE", bufs=2,
                                      name=f"pp{b}_{g}_{mi}")
                        for nn2 in range(2):
                            nc.tensor.matmul(psp[:, 512 * nn2:512 * (nn2 + 1)],
                                             avT[:, 128 * m:128 * (m + 1)],
                                             wp16[:, 512 * nn2:512 * (nn2 + 1)],
                                             start=True, stop=True)
                        ao = sb.tile([128, 1024], F32, tag="ao", bufs=4,
                                     name=f"ao{b}_{g}_{mi}")
                        if mi % 2 == 0:
                            nc.scalar.copy(ao[:], psp[:])
                        else:
                            nc.vector.tensor_copy(ao[:], psp[:])
                        nc.sync.dma_start(apart_ext[b, 128 * m:128 * (m + 1), :], ao[:])
    nc.compile()
    return nc


def _get_nc():
    if "nc" not in _CACHE:
        _CACHE["nc"] = _build()
    return _CACHE["nc"]


def kernel(x, w_attn, b_attn, w_proj, b_proj):
    global _last_in_maps
    x = np.asarray(x, dtype=np.float32)
    w_attn = np.asarray(w_attn, dtype=np.float32)
    b_attn = np.asarray(b_attn, dtype=np.float32)
    w_proj = np.asarray(w_proj, dtype=np.float32)
    b_proj = np.asarray(b_proj, dtype=np.float32)

    nc = _get_nc()
    x16t = np.ascontiguousarray(x.reshape(B * S, D).astype(np.float16).T)
    in_maps = []
    for c in range(N_CORES):
        lo, hi = PD * c, PD * (c + 1)
        wqkv = np.ascontiguousarray(np.concatenate(
            [w_attn[:, lo:hi], w_attn[:, D + lo:D + hi], w_attn[:, 2 * D + lo:2 * D + hi]],
            axis=1).astype(np.float16))
        bqkv = np.ascontiguousarray(np.stack(
            [b_attn[lo:hi], b_attn[D + lo:D + hi], b_attn[2 * D + lo:2 * D + hi]]))
        in_maps.append({
            "x16t": x16t,
            "wqkv16": wqkv,
            "bqkv": bqkv,
            "wp16": np.ascontiguousarray(w_proj[lo:hi, :].astype(np.float16)),
        })

    _last_in_maps = in_maps
    res = run_bass_kernel_spmd(nc, in_maps, list(range(N_CORES)))
    rs = res.results

    attn = np.concatenate([r["attn"] for r in rs], axis=1)          # [B, H, S, S]
    present = np.concatenate([r["present"] for r in rs], axis=2)    # [2, B, H, S, HD]
    a = rs[0]["a_part"]
    for r in rs[1:]:
        a = a + r["a_part"]
    a = a + b_proj
    return a, present, attn


# revision 24
# speedup vs baseline: 1.0105x; 1.0073x over previous
"""Trainium2 Bass kernel for GPT-style attention block (B=2, S=2048, D=1024, H=16).

Sharding: tensor-parallel over heads, 2 heads per core (8 cores).
Each core computes qkv for its heads, causal softmax attention, its partial
output projection (contracting only its 128 head-dims); host sums the 8
partial projections (+ b_proj) and concatenates head-sharded attn/present.

Precision: fp16 matmul operands (1 cyc/row, fp32 PSUM accumulation); the
softmax runs in fp32 with fp16-rounded outputs (~5e-4 rel err).
Structure: x arrives host-pre-transposed; scores are computed twice — once
row-major for the attn output (exp + normalize, exact causal widths), once
transposed (k.T@q) feeding attn@v directly. Softmax row-sums fall out of
the attn@v matmul via a ones-column appended to the stationary v tiles.
Causal upper region relies on pre-zeroed DRAM outputs.
"""
import numpy as np

import concourse.bacc as bacc
import concourse.mybir as mybir
import concourse.tile as tile
from concourse.bass_utils import run_bass_kernel_spmd
from concourse.masks import make_identity

B, S, D, H = 2, 2048, 1024, 16
HD = D // H            # 64
N_CORES = 8
HPC = H // N_CORES     # 2 heads per core
PD = HPC * HD          # 128 partition dims per core
F32 = mybir.dt.float32
F16 = mybir.dt.float16
AF = mybir.ActivationFunctionType

_CACHE = {}
_last_in_maps = None


def _build():
    nc = bacc.Bacc(None, target_bir_lowering=False)

    xt_ext = nc.declare_dram_parameter("x16t", [D, B * S], F16, isOutput=False)
    wqkv_ext = nc.declare_dram_parameter("wqkv16", [D, 3 * PD], F16, isOutput=False)
    bqkv_ext = nc.declare_dram_parameter("bqkv", [3, PD], F32, isOutput=False)
    wp_ext = nc.declare_dram_parameter("wp16", [PD, D], F16, isOutput=False)
    attn_ext = nc.declare_dram_parameter("attn", [B, HPC, S, S], F32, isOutput=True)
    pres_ext = nc.declare_dram_parameter("present", [2, B, HPC, S, HD], F32, isOutput=True)
    apart_ext = nc.declare_dram_parameter("a_part", [B, S, D], F32, isOutput=True)

    with tile.TileContext(nc) as tc:
        with (
            tc.tile_pool(name="cst", bufs=1) as cst,
            tc.tile_pool(name="sb", bufs=1) as sb,
            tc.tile_pool(name="ps", bufs=1, space="PSUM") as ps,
        ):
            # ---- constants ----
            ident = cst.tile([128, 128], F32)
            make_identity(nc, ident[:])
            ident16 = cst.tile([128, 128], F16)
            nc.vector.tensor_copy(ident16[:], ident[:])

            # row-major diag-block mask: keep where col <= row
            trimask = cst.tile([128, 128], F32)
            nc.gpsimd.memset(trimask[:], 0.0)
            nc.gpsimd.affine_select(
                out=trimask[:], in_=trimask[:],
                compare_op=mybir.AluOpType.is_ge,
                fill=-1e9, base=0,
                pattern=[[-1, 128]], channel_multiplier=1,
            )
            # transposed diag-block mask: keep where qrow-offset >= kcol-offset
            trimaskT = cst.tile([128, 128], F32)
            nc.gpsimd.memset(trimaskT[:], 0.0)
            nc.gpsimd.affine_select(
                out=trimaskT[:], in_=trimaskT[:],
                compare_op=mybir.AluOpType.is_ge,
                fill=-1e9, base=0,
                pattern=[[1, 128]], channel_multiplier=-1,
            )

            # qkv weights: 8 k-blocks x 3 col-tiles (fp16, direct DMA)
            wsb = [[None] * 3 for _ in range(8)]
            for k in range(8):
                for t in range(3):
                    wr = cst.tile([128, 128], F16, name=f"w{k}_{t}")
                    nc.sync.dma_start(wr[:], wqkv_ext[128 * k:128 * (k + 1), 128 * t:128 * (t + 1)])
                    wsb[k][t] = wr

            bq_sb = cst.tile([128, 3], F32)
            for t in range(3):
                nc.sync.dma_start(bq_sb[:, t:t + 1], bqkv_ext[t][:, None])

            wp16 = cst.tile([PD, D], F16)
            nc.sync.dma_start(wp16[:], wp_ext[:])

            for b in range(B):
                # ---- qkv projection (transposed activations direct from DRAM) ----
                qT = sb.tile([128, S], F16, tag="qT", bufs=2, name=f"qT{b}")
                kT = sb.tile([128, S], F16, tag="kT", bufs=2, name=f"kT{b}")
                vbigs = []
                for n in range(2):              # 1024-wide s-chunks
                    xts = []
                    for k in range(8):
                        xt = sb.tile([128, 1024], F16, tag="xt", bufs=12, name=f"xt{b}_{n}_{k}")
                        nc.sync.dma_start(
                            xt[:], xt_ext[128 * k:128 * (k + 1),
                                          2048 * b + 1024 * n:2048 * b + 1024 * (n + 1)])
                        xts.append(xt)

                    for t in range(3):
                        psq = ps.tile([128, 1024], F32, tag="mmE", bufs=2, name=f"pq{b}_{n}_{t}")
                        for k in range(8):
                            for half in range(2):
                                nc.tensor.matmul(psq[:, 512 * half:512 * (half + 1)],
                                                 wsb[k][t][:],
                                                 xts[k][:, 512 * half:512 * (half + 1)],
                                                 start=(k == 0), stop=(k == 7))
                        if t == 0:
                            dst = qT[:, 1024 * n:1024 * (n + 1)]
                        elif t == 1:
                            dst = kT[:, 1024 * n:1024 * (n + 1)]
                        else:
                            vt = sb.tile([128, 1024], F16, tag="vt", bufs=3, name=f"vt{b}_{n}")
                            dst = vt[:]
                        nc.vector.tensor_scalar_add(dst, psq[:], bq_sb[:, t:t + 1])

                    # k/v row-major (present outputs via casting DMA; v + ones
                    # columns also feed attn@v)
                    psk = ps.tile([128, 1024], F16, tag="mmT", bufs=2, name=f"psk{b}_{n}")
                    for r in range(8):
                        nc.tensor.transpose(psk[:, 128 * r:128 * (r + 1)],
                                            kT[:, 1024 * n + 128 * r:1024 * n + 128 * (r + 1)],
                                            ident16[:])
                    kbig = sb.tile([128, 1024], F32, tag="kbig", bufs=3, name=f"kb{b}_{n}")
                    nc.vector.tensor_copy(kbig[:], psk[:])
                    for hh2 in range(2):
                        nc.sync.dma_start(
                            pres_ext[0, b, hh2, 1024 * n:1024 * (n + 1), :]
                            .rearrange("(r p) d -> p r d", p=128),
                            kbig[:].rearrange("p (r h d) -> p r h d", r=8, h=2)[:, :, hh2, :])
                    psv = ps.tile([128, 1024], F16, tag="mmT", bufs=2, name=f"psv{b}_{n}")
                    for r in range(8):
                        nc.tensor.transpose(psv[:, 128 * r:128 * (r + 1)],
                                            vt[:, 128 * r:128 * (r + 1)], ident16[:])
                    # vbig layout per 130-col block r: [v_h0(64) | ones | v_h1(64) | ones]
                    vbig = sb.tile([128, 1040], F16, tag="vbig", bufs=4, name=f"vb{b}_{n}")
                    nc.vector.tensor_copy(
                        vbig[:].rearrange("p (k c) -> p k c", c=65)[:, :, 0:64],
                        psv[:].rearrange("p (k d) -> p k d", d=64))
                    nc.vector.memset(
                        vbig[:].rearrange("p (k c) -> p k c", c=65)[:, :, 64:65], 1.0)
                    for hh2 in range(2):
                        nc.gpsimd.dma_start(
                            pres_ext[1, b, hh2, 1024 * n:1024 * (n + 1), :]
                            .rearrange("(r p) d -> p r d", p=128),
                            vbig[:].rearrange("p (r h c) -> p r h c", h=2, c=65)[:, :, hh2, 0:64])
                    vbigs.append(vbig)

                # ---- attention ----
                avT = sb.tile([128, S], F16, tag="avT", bufs=2, name=f"avT{b}")
                for g in range(4):
                    for hh in range(2):
                        hs = 64 * hh
                        # transposed side: scoresT -> expT -> attn@v (+row sums)
                        avp = ps.tile([65, 512], F32, tag="av", bufs=2,
                                      name=f"avp{b}_{hh}_{g}")
                        njs = 4 * (g + 1)
                        for j in range(njs):
                            qoff = max(0, 128 * (j - 4 * g))
                            nw = 512 - qoff
                            pssT = ps.tile([128, 512], F32, tag="mmT", bufs=2,
                                           name=f"psT{b}_{hh}_{g}_{j}")
                            nc.tensor.matmul(
                                pssT[:, :nw],
                                kT[hs:hs + 64, 128 * j:128 * (j + 1)],
                                qT[hs:hs + 64, 512 * g + qoff:512 * (g + 1)],
                                start=True, stop=True)
                            if j >= 4 * g:
                                nc.vector.tensor_add(pssT[:, 0:128], pssT[:, 0:128],
                                                     trimaskT[:])
                            ET = sb.tile([128, 512], F16, tag="ET", bufs=8,
                                         name=f"ET{b}_{hh}_{g}_{j}")
                            nc.scalar.activation(out=ET[:, :nw], in_=pssT[:, :nw],
                                                 func=AF.Exp, scale=0.125)
                            vb = vbigs[j // 8]
                            lhsv = vb[:, 130 * (j % 8) + 65 * hh:130 * (j % 8) + 65 * hh + 65]
                            nc.tensor.matmul(avp[:, qoff:512], lhsv, ET[:, :nw],
                                             start=(j == 0), stop=(j == njs - 1))
                        # row sums -> per-row reciprocals -> broadcast numerators
                        rsT = sb.tile([1, 512], F32, tag="rsT", bufs=3,
                                      name=f"rt{b}_{hh}_{g}")
                        nc.vector.tensor_copy(rsT[:], avp[64:65, :])
                        stgp = ps.tile([128, 4], F32, tag="mmT", bufs=2,
                                       name=f"sg{b}_{hh}_{g}")
                        for r in range(4):
                            nc.tensor.transpose(stgp[:, r:r + 1],
                                                rsT[0:1, 128 * r:128 * (r + 1)],
                                                ident[0:1, 0:1])
                        stage = sb.tile([128, 4], F32, tag="stg", bufs=4,
                                        name=f"st{b}_{hh}_{g}")
                        nc.vector.reciprocal(stage[:], stgp[:])
                        rcpTp = ps.tile([1, 512], F32, tag="mmT", bufs=2,
                                        name=f"rp{b}_{hh}_{g}")
                        for r in range(4):
                            nc.tensor.transpose(rcpTp[0:1, 128 * r:128 * (r + 1)],
                                                stage[:, r:r + 1], ident[:])
                        rcpT = sb.tile([1, 512], F32, tag="rcpT", bufs=3,
                                       name=f"rr{b}_{hh}_{g}")
                        nc.vector.tensor_copy(rcpT[:], rcpTp[0:1, :])
                        rbs = sb.tile([64, 512], F32, tag="rb", bufs=3,
                                      name=f"rb{b}_{hh}_{g}")
                        nc.gpsimd.partition_broadcast(rbs[:], rcpT[:])
                        nc.vector.tensor_mul(avT[hs:hs + 64, 512 * g:512 * (g + 1)],
                                             avp[0:64, :], rbs[:])
                        # row-major side: exp with exact causal widths -> attn out
                        for r in range(4):
                            i = 4 * g + r
                            widE = 128 * (i + 1)
                            E = sb.tile([128, 2048], F16, tag="E", bufs=9,
                                        name=f"E{b}_{hh}_{i}")
                            accs = []
                            for p in range((widE + 1023) // 1024):
                                wp_ = min(1024, widE - 1024 * p)
                                pss = ps.tile([128, 1024], F32, tag="mmE", bufs=2,
                                              name=f"pss{b}_{hh}_{i}_{p}")
                                for c0 in range(0, wp_, 512):
                                    cw = min(512, wp_ - c0)
                                    nc.tensor.matmul(
                                        pss[:, c0:c0 + cw],
                                        qT[hs:hs + 64, 128 * i:128 * (i + 1)],
                                        kT[hs:hs + 64, 1024 * p + c0:1024 * p + c0 + cw],
                                        start=True, stop=True)
                                if 1024 * p + wp_ == widE:
                                    nc.vector.tensor_add(pss[:, wp_ - 128:wp_],
                                                         pss[:, wp_ - 128:wp_], trimask[:])
                                acc = sb.tile([128, 1], F32, tag="acc", bufs=8,
                                              name=f"ac{b}_{hh}_{i}_{p}")
                                nc.scalar.activation(out=E[:, 1024 * p:1024 * p + wp_],
                                                     in_=pss[:, :wp_],
                                                     func=AF.Exp, scale=0.125,
                                                     accum_out=acc[:])
                                accs.append(acc)
                            if len(accs) == 1:
                                rsum = accs[0]
                            else:
                                rsum = sb.tile([128, 1], F32, tag="rs", bufs=4,
                                               name=f"rsm{b}_{hh}_{i}")
                                nc.vector.tensor_add(rsum[:], accs[0][:], accs[1][:])
                            rcp = sb.tile([128, 1], F32, tag="rcp", bufs=8,
                                          name=f"rcE{b}_{hh}_{i}")
                            nc.vector.reciprocal(rcp[:], rsum[:])
                            nc.vector.tensor_scalar_mul(E[:, :widE], E[:, :widE],
                                                        rcp[:])
                            nc.gpsimd.dma_start(
                                attn_ext[b, hh, 128 * i:128 * (i + 1), 0:widE],
                                E[:, :widE])
                    # ---- partial projection for this row group (fp16) ----
                    for mi in range(4):
                        m = 4 * g + mi
                        psp = ps.tile([128, 1024], F32, tag="mmE", bufs=2,
                                      name=f"pp{b}_{g}_{mi}")
                        for nn2 in range(2):
                            nc.tensor.matmul(psp[:, 512 * nn2:512 * (nn2 + 1)],
                                             avT[:, 128 * m:128 * (m + 1)],
                                             wp16[:, 512 * nn2:512 * (nn2 + 1)],
                                             start=True, stop=True)
                        ao = sb.tile([128, 1024], F32, tag="ao", bufs=4,
                                     name=f"ao{b}_{g}_{mi}")
                        if mi % 2 == 0:
                            nc.scalar.copy(ao[:], psp[:])
                        else:
                            nc.vector.tensor_copy(ao[:], psp[:])
                        nc.sync.dma_start(apart_ext[b, 128 * m:128 * (m + 1), :], ao[:])
    nc.compile()
    return nc


def _get_nc():
    if "nc" not in _CACHE:
        _CACHE["nc"] = _build()
    return _CACHE["nc"]


def kernel(x, w_attn, b_attn, w_proj, b_proj):
    global _last_in_maps
    x = np.asarray(x, dtype=np.float32)
    w_attn = np.asarray(w_attn, dtype=np.float32)
    b_attn = np.asarray(b_attn, dtype=np.float32)
    w_proj = np.asarray(w_proj, dtype=np.float32)
    b_proj = np.asarray(b_proj, dtype=np.float32)

    nc = _get_nc()
    x16t = np.ascontiguousarray(x.reshape(B * S, D).astype(np.float16).T)
    in_maps = []
    for c in range(N_CORES):
        lo, hi = PD * c, PD * (c + 1)
        wqkv = np.ascontiguousarray(np.concatenate(
            [w_attn[:, lo:hi], w_attn[:, D + lo:D + hi], w_attn[:, 2 * D + lo:2 * D + hi]],
            axis=1).astype(np.float16))
        bqkv = np.ascontiguousarray(np.stack(
            [b_attn[lo:hi], b_attn[D + lo:D + hi], b_attn[2 * D + lo:2 * D + hi]]))
        in_maps.append({
            "x16t": x16t,
            "wqkv16": wqkv,
            "bqkv": bqkv,
            "wp16": np.ascontiguousarray(w_proj[lo:hi, :].astype(np.float16)),
        })

    _last_in_maps = in_maps
    res = run_bass_kernel_spmd(nc, in_maps, list(range(N_CORES)))
    rs = res.results

    attn = np.concatenate([r["attn"] for r in rs], axis=1)          # [B, H, S, S]
    present = np.concatenate([r["present"] for r in rs], axis=2)    # [2, B, H, S, HD]
    a = rs[0]["a_part"]
    for r in rs[1:]:
        a = a + r["a_part"]
    a = a + b_proj
    return a, present, attn


# revision 25
# speedup vs baseline: 1.0544x; 1.0435x over previous
"""Trainium2 Bass kernel for GPT-style attention block (B=2, S=2048, D=1024, H=16).

Sharding: tensor-parallel over heads, 2 heads per core (8 cores).
Each core computes qkv for its heads, causal softmax attention, its partial
output projection (contracting only its 128 head-dims); host sums the 8
partial projections (+ b_proj) and concatenates head-sharded attn/present.

Precision: fp16 matmul operands (1 cyc/row, fp32 PSUM accumulation); the
softmax runs in fp32 with fp16-rounded outputs (~5e-4 rel err).
Structure: x arrives host-pre-transposed; scores are computed twice — once
row-major for the attn output (exp + normalize, exact causal widths), once
transposed (k.T@q) feeding attn@v directly. Softmax row-sums fall out of
the attn@v matmul via a ones-column appended to the stationary v tiles.
Causal upper region relies on pre-zeroed DRAM outputs.
"""
import numpy as np

import concourse.bacc as bacc
import concourse.mybir as mybir
import concourse.tile as tile
from concourse.bass_utils import run_bass_kernel_spmd
from concourse.masks import make_identity

B, S, D, H = 2, 2048, 1024, 16
HD = D // H            # 64
N_CORES = 8
HPC = H // N_CORES     # 2 heads per core
PD = HPC * HD          # 128 partition dims per core
F32 = mybir.dt.float32
F16 = mybir.dt.float16
AF = mybir.ActivationFunctionType

_CACHE = {}
_last_in_maps = None


def _build():
    nc = bacc.Bacc(None, target_bir_lowering=False)

    xt_ext = nc.declare_dram_parameter("x16t", [D, B * S], F16, isOutput=False)
    wqkv_ext = nc.declare_dram_parameter("wqkv16", [D, 3 * PD], F16, isOutput=False)
    bqkv_ext = nc.declare_dram_parameter("bqkv", [3, PD], F32, isOutput=False)
    wp_ext = nc.declare_dram_parameter("wp16", [PD, D], F16, isOutput=False)
    attn_ext = nc.declare_dram_parameter("attn", [B, HPC, S, S], F32, isOutput=True)
    pres_ext = nc.declare_dram_parameter("present", [2, B, HPC, S, HD], F32, isOutput=True)
    apart_ext = nc.declare_dram_parameter("a_part", [B, S, D], F32, isOutput=True)

    with tile.TileContext(nc) as tc:
        with (
            tc.tile_pool(name="cst", bufs=1) as cst,
            tc.tile_pool(name="sb", bufs=1) as sb,
            tc.tile_pool(name="ps", bufs=1, space="PSUM") as ps,
        ):
            # ---- constants ----
            ident = cst.tile([128, 128], F32)
            make_identity(nc, ident[:])
            ident16 = cst.tile([128, 128], F16)
            nc.vector.tensor_copy(ident16[:], ident[:])

            # row-major diag-block mask: keep where col <= row
            trimask = cst.tile([128, 128], F32)
            nc.gpsimd.memset(trimask[:], 0.0)
            nc.gpsimd.affine_select(
                out=trimask[:], in_=trimask[:],
                compare_op=mybir.AluOpType.is_ge,
                fill=-1e9, base=0,
                pattern=[[-1, 128]], channel_multiplier=1,
            )
            # transposed diag-block mask: keep where qrow-offset >= kcol-offset
            trimaskT = cst.tile([128, 128], F32)
            nc.gpsimd.memset(trimaskT[:], 0.0)
            nc.gpsimd.affine_select(
                out=trimaskT[:], in_=trimaskT[:],
                compare_op=mybir.AluOpType.is_ge,
                fill=-1e9, base=0,
                pattern=[[1, 128]], channel_multiplier=-1,
            )

            # qkv weights: 8 k-blocks x 3 col-tiles (fp16, direct DMA)
            wsb = [[None] * 3 for _ in range(8)]
            for k in range(8):
                for t in range(3):
                    wr = cst.tile([128, 128], F16, name=f"w{k}_{t}")
                    nc.sync.dma_start(wr[:], wqkv_ext[128 * k:128 * (k + 1), 128 * t:128 * (t + 1)])
                    wsb[k][t] = wr

            bq_sb = cst.tile([128, 3], F32)
            for t in range(3):
                nc.sync.dma_start(bq_sb[:, t:t + 1], bqkv_ext[t][:, None])

            wp16 = cst.tile([PD, D], F16)
            nc.sync.dma_start(wp16[:], wp_ext[:])

            for b in range(B):
                # ---- qkv projection (transposed activations direct from DRAM) ----
                qT = sb.tile([128, S], F16, tag="qT", bufs=2, name=f"qT{b}")
                kT = sb.tile([128, S], F16, tag="kT", bufs=2, name=f"kT{b}")
                vbigs = []
                for n in range(2):              # 1024-wide s-chunks
                    xts = []
                    for k in range(8):
                        xt = sb.tile([128, 1024], F16, tag="xt", bufs=12, name=f"xt{b}_{n}_{k}")
                        nc.sync.dma_start(
                            xt[:], xt_ext[128 * k:128 * (k + 1),
                                          2048 * b + 1024 * n:2048 * b + 1024 * (n + 1)])
                        xts.append(xt)

                    for t in range(3):
                        psq = ps.tile([128, 1024], F32, tag="mmE", bufs=2, name=f"pq{b}_{n}_{t}")
                        for k in range(8):
                            for half in range(2):
                                nc.tensor.matmul(psq[:, 512 * half:512 * (half + 1)],
                                                 wsb[k][t][:],
                                                 xts[k][:, 512 * half:512 * (half + 1)],
                                                 start=(k == 0), stop=(k == 7))
                        if t == 0:
                            dst = qT[:, 1024 * n:1024 * (n + 1)]
                        elif t == 1:
                            dst = kT[:, 1024 * n:1024 * (n + 1)]
                        else:
                            vt = sb.tile([128, 1024], F16, tag="vt", bufs=3, name=f"vt{b}_{n}")
                            dst = vt[:]
                        nc.vector.tensor_scalar_add(dst, psq[:], bq_sb[:, t:t + 1])

                    # k/v row-major (present outputs via casting DMA; v + ones
                    # columns also feed attn@v)
                    psk = ps.tile([128, 1024], F16, tag="mmT", bufs=2, name=f"psk{b}_{n}")
                    for r in range(8):
                        nc.tensor.transpose(psk[:, 128 * r:128 * (r + 1)],
                                            kT[:, 1024 * n + 128 * r:1024 * n + 128 * (r + 1)],
                                            ident16[:])
                    kbig = sb.tile([128, 1024], F32, tag="kbig", bufs=3, name=f"kb{b}_{n}")
                    nc.vector.tensor_copy(kbig[:], psk[:])
                    for hh2 in range(2):
                        nc.sync.dma_start(
                            pres_ext[0, b, hh2, 1024 * n:1024 * (n + 1), :]
                            .rearrange("(r p) d -> p r d", p=128),
                            kbig[:].rearrange("p (r h d) -> p r h d", r=8, h=2)[:, :, hh2, :])
                    psv = ps.tile([128, 1024], F16, tag="mmT", bufs=2, name=f"psv{b}_{n}")
                    for r in range(8):
                        nc.tensor.transpose(psv[:, 128 * r:128 * (r + 1)],
                                            vt[:, 128 * r:128 * (r + 1)], ident16[:])
                    # vbig layout per 130-col block r: [v_h0(64) | ones | v_h1(64) | ones]
                    vbig = sb.tile([128, 1040], F16, tag="vbig", bufs=4, name=f"vb{b}_{n}")
                    nc.vector.tensor_copy(
                        vbig[:].rearrange("p (k c) -> p k c", c=65)[:, :, 0:64],
                        psv[:].rearrange("p (k d) -> p k d", d=64))
                    nc.vector.memset(
                        vbig[:].rearrange("p (k c) -> p k c", c=65)[:, :, 64:65], 1.0)
                    for hh2 in range(2):
                        nc.gpsimd.dma_start(
                            pres_ext[1, b, hh2, 1024 * n:1024 * (n + 1), :]
                            .rearrange("(r p) d -> p r d", p=128),
                            vbig[:].rearrange("p (r h c) -> p r h c", h=2, c=65)[:, :, hh2, 0:64])
                    vbigs.append(vbig)

                # ---- attention ----
                avT = sb.tile([128, S], F16, tag="avT", bufs=2, name=f"avT{b}")
                for g in range(4):
                    for hh in range(2):
                        hs = 64 * hh
                        # transposed side: scoresT -> expT -> attn@v (+row sums)
                        avp = ps.tile([65, 512], F32, tag="av", bufs=2,
                                      name=f"avp{b}_{hh}_{g}")
                        njs = 4 * (g + 1)
                        for j in range(njs):
                            qoff = max(0, 128 * (j - 4 * g))
                            nw = 512 - qoff
                            pssT = ps.tile([128, 512], F32, tag="mmT", bufs=2,
                                           name=f"psT{b}_{hh}_{g}_{j}")
                            nc.tensor.matmul(
                                pssT[:, :nw],
                                kT[hs:hs + 64, 128 * j:128 * (j + 1)],
                                qT[hs:hs + 64, 512 * g + qoff:512 * (g + 1)],
                                start=True, stop=True)
                            if j >= 4 * g:
                                nc.vector.tensor_add(pssT[:, 0:128], pssT[:, 0:128],
                                                     trimaskT[:])
                            ET = sb.tile([128, 512], F16, tag="ET", bufs=8,
                                         name=f"ET{b}_{hh}_{g}_{j}")
                            nc.scalar.activation(out=ET[:, :nw], in_=pssT[:, :nw],
                                                 func=AF.Exp, scale=0.125)
                            vb = vbigs[j // 8]
                            lhsv = vb[:, 130 * (j % 8) + 65 * hh:130 * (j % 8) + 65 * hh + 65]
                            nc.tensor.matmul(avp[:, qoff:512], lhsv, ET[:, :nw],
                                             start=(j == 0), stop=(j == njs - 1))
                        pass  # avT normalization moved after the row-major side
                        # row-major side: exp with exact causal widths -> attn out
                        rcps = []
                        for r in range(4):
                            i = 4 * g + r
                            widE = 128 * (i + 1)
                            E = sb.tile([128, 2048], F16, tag="E", bufs=9,
                                        name=f"E{b}_{hh}_{i}")
                            accs = []
                            for p in range((widE + 1023) // 1024):
                                wp_ = min(1024, widE - 1024 * p)
                                pss = ps.tile([128, 1024], F32, tag="mmE", bufs=2,
                                              name=f"pss{b}_{hh}_{i}_{p}")
                                for c0 in range(0, wp_, 512):
                                    cw = min(512, wp_ - c0)
                                    nc.tensor.matmul(
                                        pss[:, c0:c0 + cw],
                                        qT[hs:hs + 64, 128 * i:128 * (i + 1)],
                                        kT[hs:hs + 64, 1024 * p + c0:1024 * p + c0 + cw],
                                        start=True, stop=True)
                                if 1024 * p + wp_ == widE:
                                    nc.vector.tensor_add(pss[:, wp_ - 128:wp_],
                                                         pss[:, wp_ - 128:wp_], trimask[:])
                                acc = sb.tile([128, 1], F32, tag="acc", bufs=8,
                                              name=f"ac{b}_{hh}_{i}_{p}")
                                nc.scalar.activation(out=E[:, 1024 * p:1024 * p + wp_],
                                                     in_=pss[:, :wp_],
                                                     func=AF.Exp, scale=0.125,
                                                     accum_out=acc[:])
                                accs.append(acc)
                            if len(accs) == 1:
                                rsum = accs[0]
                            else:
                                rsum = sb.tile([128, 1], F32, tag="rs", bufs=4,
                                               name=f"rsm{b}_{hh}_{i}")
                                nc.vector.tensor_add(rsum[:], accs[0][:], accs[1][:])
                            rcp = sb.tile([128, 1], F32, tag="rcp", bufs=8,
                                          name=f"rcE{b}_{hh}_{i}")
                            nc.vector.reciprocal(rcp[:], rsum[:])
                            nc.vector.tensor_scalar_mul(E[:, :widE], E[:, :widE],
                                                        rcp[:])
                            nc.gpsimd.dma_start(
                                attn_ext[b, hh, 128 * i:128 * (i + 1), 0:widE],
                                E[:, :widE])
                            rcps.append(rcp)
                        # broadcast the E-side reciprocals to normalize attn@v
                        rcpTp = ps.tile([1, 512], F32, tag="mmT", bufs=2,
                                        name=f"rp{b}_{hh}_{g}")
                        for r in range(4):
                            nc.tensor.transpose(rcpTp[0:1, 128 * r:128 * (r + 1)],
                                                rcps[r][:], ident[:])
                        rcpT = sb.tile([1, 512], F32, tag="rcpT", bufs=3,
                                       name=f"rr{b}_{hh}_{g}")
                        nc.vector.tensor_copy(rcpT[:], rcpTp[0:1, :])
                        rbs = sb.tile([64, 512], F32, tag="rb", bufs=3,
                                      name=f"rb{b}_{hh}_{g}")
                        nc.gpsimd.partition_broadcast(rbs[:], rcpT[:])
                        nc.vector.tensor_mul(avT[hs:hs + 64, 512 * g:512 * (g + 1)],
                                             avp[0:64, :], rbs[:])
                    # ---- partial projection for this row group (fp16) ----
                    for mi in range(4):
                        m = 4 * g + mi
                        psp = ps.tile([128, 1024], F32, tag="mmE", bufs=2,
                                      name=f"pp{b}_{g}_{mi}")
                        for nn2 in range(2):
                            nc.tensor.matmul(psp[:, 512 * nn2:512 * (nn2 + 1)],
                                             avT[:, 128 * m:128 * (m + 1)],
                                             wp16[:, 512 * nn2:512 * (nn2 + 1)],
                                             start=True, stop=True)
                        ao = sb.tile([128, 1024], F32, tag="ao", bufs=4,
                                     name=f"ao{b}_{g}_{mi}")
                        if mi % 2 == 0:
                            nc.scalar.copy(ao[:], psp[:])
                        else:
                            nc.vector.tensor_copy(ao[:], psp[:])
                        nc.sync.dma_start(apart_ext[b, 128 * m:128 * (m + 1), :], ao[:])
    nc.compile()
    return nc


def _get_nc():
    if "nc" not in _CACHE:
        _CACHE["nc"] = _build()
    return _CACHE["nc"]


def kernel(x, w_attn, b_attn, w_proj, b_proj):
    global _last_in_maps
    x = np.asarray(x, dtype=np.float32)
    w_attn = np.asarray(w_attn, dtype=np.float32)
    b_attn = np.asarray(b_attn, dtype=np.float32)
    w_proj = np.asarray(w_proj, dtype=np.float32)
    b_proj = np.asarray(b_proj, dtype=np.float32)

    nc = _get_nc()
    x16t = np.ascontiguousarray(x.reshape(B * S, D).astype(np.float16).T)
    in_maps = []
    for c in range(N_CORES):
        lo, hi = PD * c, PD * (c + 1)
        wqkv = np.ascontiguousarray(np.concatenate(
            [w_attn[:, lo:hi], w_attn[:, D + lo:D + hi], w_attn[:, 2 * D + lo:2 * D + hi]],
            axis=1).astype(np.float16))
        bqkv = np.ascontiguousarray(np.stack(
            [b_attn[lo:hi], b_attn[D + lo:D + hi], b_attn[2 * D + lo:2 * D + hi]]))
        in_maps.append({
            "x16t": x16t,
            "wqkv16": wqkv,
            "bqkv": bqkv,
            "wp16": np.ascontiguousarray(w_proj[lo:hi, :].astype(np.float16)),
        })

    _last_in_maps = in_maps
    res = run_bass_kernel_spmd(nc, in_maps, list(range(N_CORES)))
    rs = res.results

    attn = np.concatenate([r["attn"] for r in rs], axis=1)          # [B, H, S, S]
    present = np.concatenate([r["present"] for r in rs], axis=2)    # [2, B, H, S, HD]
    a = rs[0]["a_part"]
    for r in rs[1:]:
        a = a + r["a_part"]
    a = a + b_proj
    return a, present, attn
